# revision 25
# baseline (speedup 1.0000x reference)
"""Trainium2 Bass kernel for nn_CELoss_4896262717859.

For each query column c = idx_node[k] of a sparse adjacency matrix (diagonal
zeroed), computes a CE-style loss over the "lower" (r < c) and "upper" (r > c)
neighbor sets:

    contrib_side(c) = [cnt>0 and poscnt==1] * (log(sum_r m exp(out_r)) - poslogit) / cnt

Only the gathered columns are ever read (host gathers them while sharding, per
the sharding hint) and duplicate idx_node entries are deduplicated, as fp8 0/1
bytes: ~3.4 MB per core instead of the 32 MB int32 full-matrix slab.

Device work per core (KC/8 columns): for each of 32 row-blocks J (256 rows),
a Double-FP8 matmul (fp8 pairs, 2 rows/lane/cycle) accumulating the per-block
partials P[J, {cnt, e_hi, e_lo}, k] into psum partitions 3J..3J+3 of a single
bank, using a zero-padded 96-wide stationary (block J's weights at column
offset 3J). The lower/upper split is NOT done on device: because columns are
sorted, lower(k) = prefix of P over J < c_k//256 plus an in-block partial the
host computes exactly from the 256-row window around the diagonal; upper(k) is
the suffix likewise. Positive-row stats (poscnt/poslogit) touch only ~2% of
rows and are host-exact. All device sums are of nonneg terms -> no
cancellation anywhere.

Weights: w0 = 1 (cnt; exact), w1 = fp8(exp(out)), w2 = fp8(exp(out) - w1)
(hi/lo split -> ~0.4% relative error on sumexp, far inside the 2e-2 gate).
"""

import numpy as np
import ml_dtypes

N = 8192
NCORES = 8
P = 128                   # partitions
NCHUNK = 8                # DMA chunks per core (1024 rows each)
TPC = 8                   # 128-row subtiles per chunk
NJ = 32                   # 256-row double-tiles (2 subtiles each)
JPC = 4                   # double-tiles per chunk
S = 3                     # stats per column: cnt, e_hi, e_lo
SW = 4                    # stat slots in the compact weight table
MH = S * NJ // 2          # 48 psum partitions per half: row 3(J%16)+s

SLAB_DEDUP = 416          # columns/core when unique(idx) fits (3328 total)
SLAB_FULL = 512           # fallback: all 4096 columns with duplicates

FP8 = ml_dtypes.float8_e4m3   # TRN FP8_EXP4 (max 240, has inf) matches this
BF16 = ml_dtypes.bfloat16

_BASS_CACHE = {}


# adjacency DMA spans, in 128-row subtile units (64 subtiles total). The
# first span is small so the first matmul's data (+ its completion receipt)
# lands early; the middle ones carry ~4 KB per partition per DMA (the
# measured DMA sweet spot); at most 8 adjacency DMAs so none stalls on
# Tile's 8 completion lanes; the last is small so the tail matmuls overlap
# the final bytes' flight.
SPANS = [(0, 2)] + [(2 + 10 * k, 12 + 10 * k) for k in range(6)] + [(62, 64)]


def _build_bass(slab):
    import concourse.tile as tile
    import concourse.mybir as mybir
    from concourse import bacc

    nc = bacc.Bacc("TRN2")
    # host pre-arranged layout: [partition, subtile*col] so each span DMA
    # moves its bytes with ONE contiguous descriptor per partition (a 3D
    # [p, t, k] AP emits per-(p,t) descriptors of `slab` bytes, which sits
    # below the 512 B DMA line-rate threshold)
    adj = nc.dram_tensor(
        "adj", [P, 64 * slab], mybir.dt.float8e4, kind="ExternalInput"
    )
    # host-prebuilt zero-padded stationary table [p, pair, J, 48]: block J's
    # weights sit at column offset 3(J%16) of a 48-wide stationary, so each
    # matmul accumulates its 3 stat rows into psum partitions of one
    # half-bank (J 0..15 -> bank A, 16..31 -> bank B; bank A is copied out
    # + DMA'd while the PE still writes bank B). The other 45 rows receive
    # exact-zero products. Engine SBUF APs must start at a 32-aligned
    # partition, so this free-dim-offset layout is the only one that both
    # engines and a wide output DMA can address.
    wpd = nc.dram_tensor(
        "wpad", [P, 2, NJ, MH], mybir.dt.float8e4, kind="ExternalInput"
    )
    # per-block partials, J-major; bf16 (cnt <= 256 stays exact)
    stats = nc.dram_tensor(
        "stats", [NJ, S, slab], mybir.dt.bfloat16, kind="ExternalOutput"
    )

    DR = mybir.MatmulPerfMode.DoubleRow

    with tile.TileContext(nc) as tc:
        with (
            tc.tile_pool(name="singles", bufs=1) as singles,
            tc.tile_pool(name="io", bufs=1) as io_pool,
            tc.tile_pool(name="psum", bufs=1, space="PSUM") as psum_pool,
        ):
            # the small first adjacency span leads the sync ring; the weight
            # table rides the scalar ring in parallel, split in two tiles so
            # the first matmul only waits on the J<16 half
            tiles = []
            for i, (a, b) in enumerate(SPANS):
                t = io_pool.tile(
                    [P, (b - a) * slab], mybir.dt.float8e4,
                    tag=f"adj{i}", name=f"adj{i}",
                )
                nc.sync.dma_start(out=t, in_=adj[:, a * slab : b * slab])
                tiles.append((t, a))
                if i == 0:
                    wpad = singles.tile([P, 2, NJ, MH], mybir.dt.float8e4)
                    nc.scalar.dma_start(out=wpad, in_=wpd[:, :, :, :])

            # pad psum tiles to full 2 KiB banks so each is bank-aligned
            pta = psum_pool.tile([MH, 512], mybir.dt.float32)
            ptb = psum_pool.tile([MH, 512], mybir.dt.float32)
            # half-split output so the first half leaves while matmuls run
            out_a = singles.tile([MH, slab], mybir.dt.bfloat16)
            out_b = singles.tile([MH, slab], mybir.dt.bfloat16)

            si = 0
            for J in range(NJ):
                sub = 2 * J
                while sub >= SPANS[si][1]:
                    si += 1
                tt, base = tiles[si]
                pt = pta if J < NJ // 2 else ptb
                Jh = J % (NJ // 2)
                off = (sub - base) * slab
                blk = tt[:, off : off + 2 * slab].rearrange(
                    "p (i k) -> p i k", i=2
                )
                # P[3(J%16)+s] += sum_i wpad[:, i, J, :].T @ blk[:, i]
                nc.tensor.matmul(
                    pt[:, 0:slab],
                    wpad[:, :, J : J + 1, :],
                    blk,
                    start=(Jh == 0),
                    stop=(Jh == NJ // 2 - 1),
                    perf_mode=DR,
                )
                if J == NJ // 2 - 1:
                    # blocks 0..15 final: evacuate + DMA out early
                    # (bank A read runs beside bank B matmul writes)
                    nc.vector.tensor_copy(out_a, pta[:, 0:slab])
                    nc.scalar.dma_start(out=stats[0 : NJ // 2], in_=out_a)

            nc.vector.tensor_copy(out_b, ptb[:, 0:slab])
            nc.scalar.dma_start(out=stats[NJ // 2 :], in_=out_b)

    nc.compile()
    return nc


def _host_prep(outputs):
    """Padded stationary table [128, 2, 32, 48] fp8: row 256J + 128i + p,
    block J's {1, e_hi, e_lo} at column offset 3*(J%16), zeros elsewhere."""
    out = np.asarray(outputs, np.float64).reshape(-1)
    e = np.exp(out)
    e_hi = e.astype(FP8)
    e_lo = (e - e_hi.astype(np.float64)).astype(FP8)

    def lay(v):  # [N] -> [P, 2, NJ]
        return v.reshape(NJ, 2, P).transpose(2, 1, 0)

    wpad = np.zeros((P, 2, NJ, MH), FP8)
    jj = np.arange(NJ)
    off = S * (jj % (NJ // 2))
    wpad[:, :, jj, off] = FP8(1.0)
    wpad[:, :, jj, off + 1] = lay(e_hi)[:, :, jj]
    wpad[:, :, jj, off + 2] = lay(e_lo)[:, :, jj]
    return np.ascontiguousarray(wpad), e


def _build_shard(node_adj, cols, core, slab):
    """fp8 gathered columns, [partition, subtile*col] layout."""
    cc = cols[core * slab : (core + 1) * slab]
    A8 = (node_adj[:, cc] != 0).astype(FP8)  # [N, slab], 0/1 exact
    return np.ascontiguousarray(
        A8.reshape(64, P, slab).transpose(1, 0, 2).reshape(P, 64 * slab)
    )


def _prepare(node_adj, idx, outputs):
    """Choose dedup vs full columns; build per-core in_maps + combine ctx."""
    idxu = np.unique(idx)
    if idxu.size <= SLAB_DEDUP * NCORES:
        slab = SLAB_DEDUP
        cols = np.concatenate(
            [idxu, np.full(slab * NCORES - idxu.size, idxu[0], np.int64)]
        )
        mapk = np.searchsorted(idxu, idx)
    else:
        slab = SLAB_FULL
        cols = idx
        mapk = np.arange(idx.size)
    wpad, e = _host_prep(outputs)
    in_maps = [
        {"adj": _build_shard(node_adj, cols, c, slab), "wpad": wpad}
        for c in range(NCORES)
    ]
    return in_maps, slab, cols, mapk, e


def _sim_stats(in_maps, slab):
    """Numpy emulation of the device kernel (same quantized inputs)."""
    outs = []
    for m in in_maps:
        adj = m["adj"].astype(np.float32)  # [p, t*slab]
        w = m["wpad"].astype(np.float32)   # [p, i, J, 48]
        st = np.zeros((NJ, S, slab), np.float32)
        for J in range(NJ):
            off = S * (J % (NJ // 2))
            for i in range(2):
                t = 2 * J + i
                blk = adj[:, t * slab : (t + 1) * slab]           # [p, k]
                st[J] += w[:, i, J, off : off + S].T @ blk        # [S, k]
        outs.append(st.astype(BF16))
    return outs


def _ensure_axon_hooks_stub():
    """bass_utils imports antenv.axon_hooks when tracing is requested via
    env; the module is absent on some images. Provide a no-op stub so the
    import never crashes (hook=None -> bass_utils skips tracing)."""
    import sys
    import types

    try:
        import antenv.axon_hooks  # noqa: F401
    except ImportError:
        mod = types.ModuleType("antenv.axon_hooks")
        state = {"hook": None}
        mod.set_axon_ntff_profile_hook = lambda h: state.__setitem__("hook", h)
        mod.get_axon_ntff_profile_hook = lambda: state["hook"]
        sys.modules["antenv.axon_hooks"] = mod


def _device_stats(in_maps, slab):
    _ensure_axon_hooks_stub()
    from concourse.bass_utils import run_bass_kernel_spmd

    if slab not in _BASS_CACHE:
        _BASS_CACHE[slab] = _build_bass(slab)
    last_exc = None
    for attempt in range(4):
        try:
            res = run_bass_kernel_spmd(
                _BASS_CACHE[slab], in_maps, core_ids=list(range(NCORES))
            )
            return [r["stats"] for r in res.results]
        except Exception as e:  # transient NRT/accelerator hiccups
            last_exc = e
            try:
                # a fresh PJRT client usually recovers a transiently
                # "unrecoverable" accelerator; mirrors a process restart
                import jax
                import jax.extend.backend as _jeb

                jax.clear_caches()
                _jeb.clear_backends()
            except Exception:
                pass
            import time

            time.sleep(2.0 * (attempt + 1))
    raise last_exc


def _combine(stats_list, node_adj, outputs, targets, cols, mapk, e):
    """Per-core [NJ, S, slab] partials -> scalar loss (f64 math)."""
    out = np.asarray(outputs, np.float64).reshape(-1)
    Pf = np.concatenate(
        [np.asarray(s, np.float64) for s in stats_list], axis=2
    )  # [NJ, S, KC]
    KC = Pf.shape[2]
    cnt_P = Pf[:, 0, :]
    se_P = Pf[:, 1, :] + Pf[:, 2, :]

    kk = np.arange(KC)
    zero = np.zeros((1, KC))
    cum_cnt = np.concatenate([zero, np.cumsum(cnt_P, axis=0)], axis=0)  # [NJ+1, KC]
    cum_se = np.concatenate([zero, np.cumsum(se_P, axis=0)], axis=0)

    t2 = cols // 256
    pre_cnt = cum_cnt[t2, kk]
    pre_se = cum_se[t2, kk]
    suf_cnt = cum_cnt[NJ] - cum_cnt[t2 + 1, kk]
    suf_se = cum_se[NJ] - cum_se[t2 + 1, kk]

    # exact in-block window (256 rows around the diagonal crossover)
    d = (cols % 256).astype(np.int64)
    rows = (cols - d)[:, None] + np.arange(256)[None, :]     # [KC, 256]
    W = node_adj[rows, cols[:, None]] != 0
    dr = np.arange(256)[None, :]
    W &= dr != d[:, None]                                    # drop diagonal
    e_win = e[rows]
    wlow = W & (dr < d[:, None])
    wup = W & (dr > d[:, None])
    lower_cnt = pre_cnt + wlow.sum(1)
    upper_cnt = suf_cnt + wup.sum(1)
    lower_se = pre_se + (e_win * wlow).sum(1)
    upper_se = suf_se + (e_win * wup).sum(1)

    # exact positive-row stats (~2% of rows)
    prows = np.flatnonzero(np.asarray(targets).reshape(-1) != 0)
    Ap = node_adj[np.ix_(prows, cols)] != 0                  # [npos, KC]
    Ap &= prows[:, None] != cols[None, :]
    plow = prows[:, None] < cols[None, :]
    poscnt_low = (Ap & plow).sum(0)
    poscnt_up = (Ap & ~plow).sum(0)
    poslogit_low = (out[prows, None] * (Ap & plow)).sum(0)
    poslogit_up = (out[prows, None] * (Ap & ~plow)).sum(0)

    def side(cnt, se, poscnt, poslogit):
        valid = (poscnt == 1) & (cnt > 0.5)
        lse = np.log(np.where(valid, np.maximum(se, 1e-300), 1.0))
        return np.where(valid, (lse - poslogit) / np.maximum(cnt, 1.0), 0.0)

    contrib = side(lower_cnt, lower_se, poscnt_low, poslogit_low) + side(
        upper_cnt, upper_se, poscnt_up, poslogit_up
    )
    return np.float32(contrib[mapk].sum())


def kernel(outputs, targets, node_adj, idx_node, _simulate=False):
    node_adj = np.asarray(node_adj)
    idx = np.asarray(idx_node).reshape(-1).astype(np.int64)
    in_maps, slab, cols, mapk, e = _prepare(node_adj, idx, outputs)
    stats = _sim_stats(in_maps, slab) if _simulate else _device_stats(in_maps, slab)
    return _combine(stats, node_adj, outputs, targets, cols, mapk, e)


# revision 29
# speedup vs baseline: 1.0572x; 1.0572x over previous
"""Trainium2 Bass kernel for nn_CELoss_4896262717859.

For each query column c = idx_node[k] of a sparse adjacency matrix (diagonal
zeroed), computes a CE-style loss over the "lower" (r < c) and "upper" (r > c)
neighbor sets:

    contrib_side(c) = [cnt>0 and poscnt==1] * (log(sum_r m exp(out_r)) - poslogit) / cnt

Only the gathered columns are ever read (host gathers them while sharding, per
the sharding hint) and duplicate idx_node entries are deduplicated, as fp8 0/1
bytes: ~3.4 MB per core instead of the 32 MB int32 full-matrix slab.

Device work per core (KC/8 columns): for each of 32 row-blocks J (256 rows),
a Double-FP8 matmul (fp8 pairs, 2 rows/lane/cycle) accumulating the per-block
partials P[J, {cnt, e_hi, e_lo}, k] into psum partitions 3J..3J+3 of a single
bank, using a zero-padded 96-wide stationary (block J's weights at column
offset 3J). The lower/upper split is NOT done on device: because columns are
sorted, lower(k) = prefix of P over J < c_k//256 plus an in-block partial the
host computes exactly from the 256-row window around the diagonal; upper(k) is
the suffix likewise. Positive-row stats (poscnt/poslogit) touch only ~2% of
rows and are host-exact. All device sums are of nonneg terms -> no
cancellation anywhere.

Weights: w0 = 1 (cnt; exact), w1 = fp8(exp(out)), w2 = fp8(exp(out) - w1)
(hi/lo split -> ~0.4% relative error on sumexp, far inside the 2e-2 gate).
"""

import numpy as np
import ml_dtypes

N = 8192
NCORES = 8
P = 128                   # partitions
NCHUNK = 8                # DMA chunks per core (1024 rows each)
TPC = 8                   # 128-row subtiles per chunk
NJ = 32                   # 256-row double-tiles (2 subtiles each)
JPC = 4                   # double-tiles per chunk
S = 3                     # stats per column: cnt, e_hi, e_lo
SW = 4                    # stat slots in the compact weight table
MH = S * NJ // 2          # 48 psum partitions per half: row 3(J%16)+s

SLAB_DEDUP = 416          # columns/core when unique(idx) fits (3328 total)
SLAB_FULL = 512           # fallback: all 4096 columns with duplicates

FP8 = ml_dtypes.float8_e4m3   # TRN FP8_EXP4 (max 240, has inf) matches this
BF16 = ml_dtypes.bfloat16

_BASS_CACHE = {}


# adjacency DMA spans, in 128-row subtile units (64 subtiles total). The
# first span is small so the first matmul's data (+ its completion receipt)
# lands early; the middle ones carry ~4 KB per partition per DMA (the
# measured DMA sweet spot); at most 8 adjacency DMAs so none stalls on
# Tile's 8 completion lanes; the last is small so the tail matmuls overlap
# the final bytes' flight.
SPANS = [(0, 2)] + [(2 + 10 * k, 12 + 10 * k) for k in range(6)] + [(62, 64)]


def _build_bass(slab):
    import concourse.tile as tile
    import concourse.mybir as mybir
    from concourse import bacc

    nc = bacc.Bacc("TRN2")
    # host pre-arranged SPAN-MAJOR layout: each span's [P, span*slab] block
    # is one fully contiguous DRAM region, so its DMA reads one sequential
    # burst (column-sliced layouts scatter each chunk into 128 strided 4 KB
    # reads and lose ~25% of HBM bandwidth)
    adj = nc.dram_tensor(
        "adj", [64 * P * slab], mybir.dt.float8e4, kind="ExternalInput"
    )
    # host-prebuilt zero-padded stationary table [p, pair, J, 48]: block J's
    # weights sit at column offset 3(J%16) of a 48-wide stationary, so each
    # matmul accumulates its 3 stat rows into psum partitions of one
    # half-bank (J 0..15 -> bank A, 16..31 -> bank B; bank A is copied out
    # + DMA'd while the PE still writes bank B). The other 45 rows receive
    # exact-zero products. Engine SBUF APs must start at a 32-aligned
    # partition, so this free-dim-offset layout is the only one that both
    # engines and a wide output DMA can address.
    wpd = nc.dram_tensor(
        "wpad", [P, 2, NJ, MH], mybir.dt.float8e4, kind="ExternalInput"
    )
    # per-block partials, J-major; bf16 (cnt <= 256 stays exact)
    stats = nc.dram_tensor(
        "stats", [NJ, S, slab], mybir.dt.bfloat16, kind="ExternalOutput"
    )

    DR = mybir.MatmulPerfMode.DoubleRow

    with tile.TileContext(nc) as tc:
        with (
            tc.tile_pool(name="singles", bufs=1) as singles,
            tc.tile_pool(name="io", bufs=1) as io_pool,
            tc.tile_pool(name="psum", bufs=1, space="PSUM") as psum_pool,
        ):
            # the small first adjacency span leads the sync ring; the weight
            # table rides the scalar ring in parallel, split in two tiles so
            # the first matmul only waits on the J<16 half
            tiles = []
            for i, (a, b) in enumerate(SPANS):
                t = io_pool.tile(
                    [P, (b - a) * slab], mybir.dt.float8e4,
                    tag=f"adj{i}", name=f"adj{i}",
                )
                src = adj[a * P * slab : b * P * slab].rearrange(
                    "(p x) -> p x", p=P
                )
                nc.sync.dma_start(out=t, in_=src)
                tiles.append((t, a))
                if i == 0:
                    wpad = singles.tile([P, 2, NJ, MH], mybir.dt.float8e4)
                    nc.scalar.dma_start(out=wpad, in_=wpd[:, :, :, :])

            # pad psum tiles to full 2 KiB banks so each is bank-aligned
            pta = psum_pool.tile([MH, 512], mybir.dt.float32)
            ptb = psum_pool.tile([MH, 512], mybir.dt.float32)
            # half-split output so the first half leaves while matmuls run
            out_a = singles.tile([MH, slab], mybir.dt.bfloat16)
            out_b = singles.tile([MH, slab], mybir.dt.bfloat16)

            si = 0
            for J in range(NJ):
                sub = 2 * J
                while sub >= SPANS[si][1]:
                    si += 1
                tt, base = tiles[si]
                pt = pta if J < NJ // 2 else ptb
                Jh = J % (NJ // 2)
                off = (sub - base) * slab
                blk = tt[:, off : off + 2 * slab].rearrange(
                    "p (i k) -> p i k", i=2
                )
                # P[3(J%16)+s] += sum_i wpad[:, i, J, :].T @ blk[:, i]
                nc.tensor.matmul(
                    pt[:, 0:slab],
                    wpad[:, :, J : J + 1, :],
                    blk,
                    start=(Jh == 0),
                    stop=(Jh == NJ // 2 - 1),
                    perf_mode=DR,
                )
                if J == NJ // 2 - 1:
                    # blocks 0..15 final: evacuate + DMA out early
                    # (bank A read runs beside bank B matmul writes)
                    nc.vector.tensor_copy(out_a, pta[:, 0:slab])
                    nc.scalar.dma_start(out=stats[0 : NJ // 2], in_=out_a)

            nc.vector.tensor_copy(out_b, ptb[:, 0:slab])
            nc.scalar.dma_start(out=stats[NJ // 2 :], in_=out_b)

    nc.compile()
    return nc


def _host_prep(outputs):
    """Padded stationary table [128, 2, 32, 48] fp8: row 256J + 128i + p,
    block J's {1, e_hi, e_lo} at column offset 3*(J%16), zeros elsewhere."""
    out = np.asarray(outputs, np.float64).reshape(-1)
    e = np.exp(out)
    e_hi = e.astype(FP8)
    e_lo = (e - e_hi.astype(np.float64)).astype(FP8)

    def lay(v):  # [N] -> [P, 2, NJ]
        return v.reshape(NJ, 2, P).transpose(2, 1, 0)

    wpad = np.zeros((P, 2, NJ, MH), FP8)
    jj = np.arange(NJ)
    off = S * (jj % (NJ // 2))
    wpad[:, :, jj, off] = FP8(1.0)
    wpad[:, :, jj, off + 1] = lay(e_hi)[:, :, jj]
    wpad[:, :, jj, off + 2] = lay(e_lo)[:, :, jj]
    return np.ascontiguousarray(wpad), e


def _build_shard(node_adj, cols, core, slab):
    """fp8 gathered columns, span-major [span][partition][subtile*col]."""
    cc = cols[core * slab : (core + 1) * slab]
    A8 = (node_adj[:, cc] != 0).astype(FP8)  # [N, slab], 0/1 exact
    sub = A8.reshape(64, P, slab)            # [subtile, p, k]
    blocks = [
        sub[a:b].transpose(1, 0, 2).reshape(-1) for a, b in SPANS
    ]
    return np.ascontiguousarray(np.concatenate(blocks))


def _prepare(node_adj, idx, outputs):
    """Choose dedup vs full columns; build per-core in_maps + combine ctx."""
    idxu = np.unique(idx)
    if idxu.size <= SLAB_DEDUP * NCORES:
        slab = SLAB_DEDUP
        cols = np.concatenate(
            [idxu, np.full(slab * NCORES - idxu.size, idxu[0], np.int64)]
        )
        mapk = np.searchsorted(idxu, idx)
    else:
        slab = SLAB_FULL
        cols = idx
        mapk = np.arange(idx.size)
    wpad, e = _host_prep(outputs)
    in_maps = [
        {"adj": _build_shard(node_adj, cols, c, slab), "wpad": wpad}
        for c in range(NCORES)
    ]
    return in_maps, slab, cols, mapk, e


def _sim_stats(in_maps, slab):
    """Numpy emulation of the device kernel (same quantized inputs)."""
    outs = []
    for m in in_maps:
        flat = m["adj"].astype(np.float32)  # span-major flat
        sub = np.zeros((64, P, slab), np.float32)
        for a, b in SPANS:
            sub[a:b] = (
                flat[a * P * slab : b * P * slab]
                .reshape(P, b - a, slab)
                .transpose(1, 0, 2)
            )
        w = m["wpad"].astype(np.float32)   # [p, i, J, 48]
        st = np.zeros((NJ, S, slab), np.float32)
        for J in range(NJ):
            off = S * (J % (NJ // 2))
            for i in range(2):
                blk = sub[2 * J + i]                              # [p, k]
                st[J] += w[:, i, J, off : off + S].T @ blk        # [S, k]
        outs.append(st.astype(BF16))
    return outs


def _ensure_axon_hooks_stub():
    """bass_utils imports antenv.axon_hooks when tracing is requested via
    env; the module is absent on some images. Provide a no-op stub so the
    import never crashes (hook=None -> bass_utils skips tracing)."""
    import sys
    import types

    try:
        import antenv.axon_hooks  # noqa: F401
    except ImportError:
        mod = types.ModuleType("antenv.axon_hooks")
        state = {"hook": None}
        mod.set_axon_ntff_profile_hook = lambda h: state.__setitem__("hook", h)
        mod.get_axon_ntff_profile_hook = lambda: state["hook"]
        sys.modules["antenv.axon_hooks"] = mod


def _device_stats(in_maps, slab):
    _ensure_axon_hooks_stub()
    from concourse.bass_utils import run_bass_kernel_spmd

    if slab not in _BASS_CACHE:
        _BASS_CACHE[slab] = _build_bass(slab)
    last_exc = None
    for attempt in range(4):
        try:
            res = run_bass_kernel_spmd(
                _BASS_CACHE[slab], in_maps, core_ids=list(range(NCORES))
            )
            return [r["stats"] for r in res.results]
        except Exception as e:  # transient NRT/accelerator hiccups
            last_exc = e
            try:
                # a fresh PJRT client usually recovers a transiently
                # "unrecoverable" accelerator; mirrors a process restart
                import jax
                import jax.extend.backend as _jeb

                jax.clear_caches()
                _jeb.clear_backends()
            except Exception:
                pass
            import time

            time.sleep(2.0 * (attempt + 1))
    raise last_exc


def _combine(stats_list, node_adj, outputs, targets, cols, mapk, e):
    """Per-core [NJ, S, slab] partials -> scalar loss (f64 math)."""
    out = np.asarray(outputs, np.float64).reshape(-1)
    Pf = np.concatenate(
        [np.asarray(s, np.float64) for s in stats_list], axis=2
    )  # [NJ, S, KC]
    KC = Pf.shape[2]
    cnt_P = Pf[:, 0, :]
    se_P = Pf[:, 1, :] + Pf[:, 2, :]

    kk = np.arange(KC)
    zero = np.zeros((1, KC))
    cum_cnt = np.concatenate([zero, np.cumsum(cnt_P, axis=0)], axis=0)  # [NJ+1, KC]
    cum_se = np.concatenate([zero, np.cumsum(se_P, axis=0)], axis=0)

    t2 = cols // 256
    pre_cnt = cum_cnt[t2, kk]
    pre_se = cum_se[t2, kk]
    suf_cnt = cum_cnt[NJ] - cum_cnt[t2 + 1, kk]
    suf_se = cum_se[NJ] - cum_se[t2 + 1, kk]

    # exact in-block window (256 rows around the diagonal crossover)
    d = (cols % 256).astype(np.int64)
    rows = (cols - d)[:, None] + np.arange(256)[None, :]     # [KC, 256]
    W = node_adj[rows, cols[:, None]] != 0
    dr = np.arange(256)[None, :]
    W &= dr != d[:, None]                                    # drop diagonal
    e_win = e[rows]
    wlow = W & (dr < d[:, None])
    wup = W & (dr > d[:, None])
    lower_cnt = pre_cnt + wlow.sum(1)
    upper_cnt = suf_cnt + wup.sum(1)
    lower_se = pre_se + (e_win * wlow).sum(1)
    upper_se = suf_se + (e_win * wup).sum(1)

    # exact positive-row stats (~2% of rows)
    prows = np.flatnonzero(np.asarray(targets).reshape(-1) != 0)
    Ap = node_adj[np.ix_(prows, cols)] != 0                  # [npos, KC]
    Ap &= prows[:, None] != cols[None, :]
    plow = prows[:, None] < cols[None, :]
    poscnt_low = (Ap & plow).sum(0)
    poscnt_up = (Ap & ~plow).sum(0)
    poslogit_low = (out[prows, None] * (Ap & plow)).sum(0)
    poslogit_up = (out[prows, None] * (Ap & ~plow)).sum(0)

    def side(cnt, se, poscnt, poslogit):
        valid = (poscnt == 1) & (cnt > 0.5)
        lse = np.log(np.where(valid, np.maximum(se, 1e-300), 1.0))
        return np.where(valid, (lse - poslogit) / np.maximum(cnt, 1.0), 0.0)

    contrib = side(lower_cnt, lower_se, poscnt_low, poslogit_low) + side(
        upper_cnt, upper_se, poscnt_up, poslogit_up
    )
    return np.float32(contrib[mapk].sum())


def kernel(outputs, targets, node_adj, idx_node, _simulate=False):
    node_adj = np.asarray(node_adj)
    idx = np.asarray(idx_node).reshape(-1).astype(np.int64)
    in_maps, slab, cols, mapk, e = _prepare(node_adj, idx, outputs)
    stats = _sim_stats(in_maps, slab) if _simulate else _device_stats(in_maps, slab)
    return _combine(stats, node_adj, outputs, targets, cols, mapk, e)


# revision 30
# speedup vs baseline: 1.1286x; 1.0676x over previous
"""Trainium2 Bass kernel for nn_CELoss_4896262717859.

For each query column c = idx_node[k] of a sparse adjacency matrix (diagonal
zeroed), computes a CE-style loss over the "lower" (r < c) and "upper" (r > c)
neighbor sets:

    contrib_side(c) = [cnt>0 and poscnt==1] * (log(sum_r m exp(out_r)) - poslogit) / cnt

Only the gathered columns are ever read (host gathers them while sharding, per
the sharding hint) and duplicate idx_node entries are deduplicated, as fp8 0/1
bytes: ~3.4 MB per core instead of the 32 MB int32 full-matrix slab.

Device work per core (KC/8 columns): for each of 32 row-blocks J (256 rows),
a Double-FP8 matmul (fp8 pairs, 2 rows/lane/cycle) accumulating the per-block
partials P[J, {cnt, e_hi, e_lo}, k] into psum partitions 3J..3J+3 of a single
bank, using a zero-padded 96-wide stationary (block J's weights at column
offset 3J). The lower/upper split is NOT done on device: because columns are
sorted, lower(k) = prefix of P over J < c_k//256 plus an in-block partial the
host computes exactly from the 256-row window around the diagonal; upper(k) is
the suffix likewise. Positive-row stats (poscnt/poslogit) touch only ~2% of
rows and are host-exact. All device sums are of nonneg terms -> no
cancellation anywhere.

Weights: w0 = 1 (cnt; exact), w1 = fp8(exp(out)), w2 = fp8(exp(out) - w1)
(hi/lo split -> ~0.4% relative error on sumexp, far inside the 2e-2 gate).
"""

import numpy as np
import ml_dtypes

N = 8192
NCORES = 8
P = 128                   # partitions
NCHUNK = 8                # DMA chunks per core (1024 rows each)
TPC = 8                   # 128-row subtiles per chunk
NJ = 32                   # 256-row double-tiles (2 subtiles each)
JPC = 4                   # double-tiles per chunk
S = 3                     # stats per column: cnt, e_hi, e_lo
SW = 4                    # stat slots in the compact weight table
MH = S * NJ // 2          # 48 psum partitions per half: row 3(J%16)+s

SLAB_DEDUP = 416          # columns/core when unique(idx) fits (3328 total)
SLAB_FULL = 512           # fallback: all 4096 columns with duplicates

FP8 = ml_dtypes.float8_e4m3   # TRN FP8_EXP4 (max 240, has inf) matches this
BF16 = ml_dtypes.bfloat16

_BASS_CACHE = {}


# adjacency DMA spans, in 128-row subtile units (64 subtiles total). The
# first span is small so the first matmul's data (+ its completion receipt)
# lands early; the middle ones carry ~4 KB per partition per DMA (the
# measured DMA sweet spot); at most 8 adjacency DMAs so none stalls on
# Tile's 8 completion lanes; the last is small so the tail matmuls overlap
# the final bytes' flight.
SPANS = [(8 * k, 8 * k + 8) for k in range(7)] + [
    (56, 58), (58, 60), (60, 62), (62, 64)
]


def _build_bass(slab):
    import concourse.tile as tile
    import concourse.mybir as mybir
    from concourse import bacc

    nc = bacc.Bacc("TRN2")
    # host pre-arranged SPAN-MAJOR layout: each span's [P, span*slab] block
    # is one fully contiguous DRAM region, so its DMA reads one sequential
    # burst (column-sliced layouts scatter each chunk into 128 strided 4 KB
    # reads and lose ~25% of HBM bandwidth)
    adj = nc.dram_tensor(
        "adj", [64 * P * slab], mybir.dt.float8e4, kind="ExternalInput"
    )
    # host-prebuilt zero-padded stationary table [p, pair, J, 48]: block J's
    # weights sit at column offset 3(J%16) of a 48-wide stationary, so each
    # matmul accumulates its 3 stat rows into psum partitions of one
    # half-bank (J 0..15 -> bank A, 16..31 -> bank B; bank A is copied out
    # + DMA'd while the PE still writes bank B). The other 45 rows receive
    # exact-zero products. Engine SBUF APs must start at a 32-aligned
    # partition, so this free-dim-offset layout is the only one that both
    # engines and a wide output DMA can address.
    wpd = nc.dram_tensor(
        "wpad", [P, 2, NJ, MH], mybir.dt.float8e4, kind="ExternalInput"
    )
    # per-block partials, J-major; bf16 (cnt <= 256 stays exact)
    stats = nc.dram_tensor(
        "stats", [NJ, S, slab], mybir.dt.bfloat16, kind="ExternalOutput"
    )

    DR = mybir.MatmulPerfMode.DoubleRow

    with tile.TileContext(nc) as tc:
        with (
            tc.tile_pool(name="singles", bufs=1) as singles,
            tc.tile_pool(name="io", bufs=1) as io_pool,
            tc.tile_pool(name="psum", bufs=1, space="PSUM") as psum_pool,
        ):
            # the small first adjacency span leads the sync ring; the weight
            # table rides the scalar ring in parallel, split in two tiles so
            # the first matmul only waits on the J<16 half
            tiles = []
            for i, (a, b) in enumerate(SPANS):
                t = io_pool.tile(
                    [P, (b - a) * slab], mybir.dt.float8e4,
                    tag=f"adj{i}", name=f"adj{i}",
                )
                src = adj[a * P * slab : b * P * slab].rearrange(
                    "(p x) -> p x", p=P
                )
                nc.sync.dma_start(out=t, in_=src)
                tiles.append((t, a))
                if i == 0:
                    wpad = singles.tile([P, 2, NJ, MH], mybir.dt.float8e4)
                    nc.scalar.dma_start(out=wpad, in_=wpd[:, :, :, :])

            # pad psum tiles to full 2 KiB banks so each is bank-aligned
            pta = psum_pool.tile([MH, 512], mybir.dt.float32)
            ptb = psum_pool.tile([MH, 512], mybir.dt.float32)
            # half-split output so the first half leaves while matmuls run
            out_a = singles.tile([MH, slab], mybir.dt.bfloat16)
            out_b = singles.tile([MH, slab], mybir.dt.bfloat16)

            si = 0
            for J in range(NJ):
                sub = 2 * J
                while sub >= SPANS[si][1]:
                    si += 1
                tt, base = tiles[si]
                pt = pta if J < NJ // 2 else ptb
                Jh = J % (NJ // 2)
                off = (sub - base) * slab
                blk = tt[:, off : off + 2 * slab].rearrange(
                    "p (i k) -> p i k", i=2
                )
                # P[3(J%16)+s] += sum_i wpad[:, i, J, :].T @ blk[:, i]
                nc.tensor.matmul(
                    pt[:, 0:slab],
                    wpad[:, :, J : J + 1, :],
                    blk,
                    start=(Jh == 0),
                    stop=(Jh == NJ // 2 - 1),
                    perf_mode=DR,
                )
                if J == NJ // 2 - 1:
                    # blocks 0..15 final: evacuate + DMA out early
                    # (bank A read runs beside bank B matmul writes)
                    nc.vector.tensor_copy(out_a, pta[:, 0:slab])
                    nc.scalar.dma_start(out=stats[0 : NJ // 2], in_=out_a)

            nc.vector.tensor_copy(out_b, ptb[:, 0:slab])
            nc.scalar.dma_start(out=stats[NJ // 2 :], in_=out_b)

    nc.compile()
    return nc


def _host_prep(outputs):
    """Padded stationary table [128, 2, 32, 48] fp8: row 256J + 128i + p,
    block J's {1, e_hi, e_lo} at column offset 3*(J%16), zeros elsewhere."""
    out = np.asarray(outputs, np.float64).reshape(-1)
    e = np.exp(out)
    e_hi = e.astype(FP8)
    e_lo = (e - e_hi.astype(np.float64)).astype(FP8)

    def lay(v):  # [N] -> [P, 2, NJ]
        return v.reshape(NJ, 2, P).transpose(2, 1, 0)

    wpad = np.zeros((P, 2, NJ, MH), FP8)
    jj = np.arange(NJ)
    off = S * (jj % (NJ // 2))
    wpad[:, :, jj, off] = FP8(1.0)
    wpad[:, :, jj, off + 1] = lay(e_hi)[:, :, jj]
    wpad[:, :, jj, off + 2] = lay(e_lo)[:, :, jj]
    return np.ascontiguousarray(wpad), e


def _build_shard(node_adj, cols, core, slab):
    """fp8 gathered columns, span-major [span][partition][subtile*col]."""
    cc = cols[core * slab : (core + 1) * slab]
    A8 = (node_adj[:, cc] != 0).astype(FP8)  # [N, slab], 0/1 exact
    sub = A8.reshape(64, P, slab)            # [subtile, p, k]
    blocks = [
        sub[a:b].transpose(1, 0, 2).reshape(-1) for a, b in SPANS
    ]
    return np.ascontiguousarray(np.concatenate(blocks))


def _prepare(node_adj, idx, outputs):
    """Choose dedup vs full columns; build per-core in_maps + combine ctx."""
    idxu = np.unique(idx)
    if idxu.size <= SLAB_DEDUP * NCORES:
        slab = SLAB_DEDUP
        cols = np.concatenate(
            [idxu, np.full(slab * NCORES - idxu.size, idxu[0], np.int64)]
        )
        mapk = np.searchsorted(idxu, idx)
    else:
        slab = SLAB_FULL
        cols = idx
        mapk = np.arange(idx.size)
    wpad, e = _host_prep(outputs)
    in_maps = [
        {"adj": _build_shard(node_adj, cols, c, slab), "wpad": wpad}
        for c in range(NCORES)
    ]
    return in_maps, slab, cols, mapk, e


def _sim_stats(in_maps, slab):
    """Numpy emulation of the device kernel (same quantized inputs)."""
    outs = []
    for m in in_maps:
        flat = m["adj"].astype(np.float32)  # span-major flat
        sub = np.zeros((64, P, slab), np.float32)
        for a, b in SPANS:
            sub[a:b] = (
                flat[a * P * slab : b * P * slab]
                .reshape(P, b - a, slab)
                .transpose(1, 0, 2)
            )
        w = m["wpad"].astype(np.float32)   # [p, i, J, 48]
        st = np.zeros((NJ, S, slab), np.float32)
        for J in range(NJ):
            off = S * (J % (NJ // 2))
            for i in range(2):
                blk = sub[2 * J + i]                              # [p, k]
                st[J] += w[:, i, J, off : off + S].T @ blk        # [S, k]
        outs.append(st.astype(BF16))
    return outs


def _ensure_axon_hooks_stub():
    """bass_utils imports antenv.axon_hooks when tracing is requested via
    env; the module is absent on some images. Provide a no-op stub so the
    import never crashes (hook=None -> bass_utils skips tracing)."""
    import sys
    import types

    try:
        import antenv.axon_hooks  # noqa: F401
    except ImportError:
        mod = types.ModuleType("antenv.axon_hooks")
        state = {"hook": None}
        mod.set_axon_ntff_profile_hook = lambda h: state.__setitem__("hook", h)
        mod.get_axon_ntff_profile_hook = lambda: state["hook"]
        sys.modules["antenv.axon_hooks"] = mod


def _device_stats(in_maps, slab):
    _ensure_axon_hooks_stub()
    from concourse.bass_utils import run_bass_kernel_spmd

    if slab not in _BASS_CACHE:
        _BASS_CACHE[slab] = _build_bass(slab)
    last_exc = None
    for attempt in range(4):
        try:
            res = run_bass_kernel_spmd(
                _BASS_CACHE[slab], in_maps, core_ids=list(range(NCORES))
            )
            return [r["stats"] for r in res.results]
        except Exception as e:  # transient NRT/accelerator hiccups
            last_exc = e
            try:
                # a fresh PJRT client usually recovers a transiently
                # "unrecoverable" accelerator; mirrors a process restart
                import jax
                import jax.extend.backend as _jeb

                jax.clear_caches()
                _jeb.clear_backends()
            except Exception:
                pass
            import time

            time.sleep(2.0 * (attempt + 1))
    raise last_exc


def _combine(stats_list, node_adj, outputs, targets, cols, mapk, e):
    """Per-core [NJ, S, slab] partials -> scalar loss (f64 math)."""
    out = np.asarray(outputs, np.float64).reshape(-1)
    Pf = np.concatenate(
        [np.asarray(s, np.float64) for s in stats_list], axis=2
    )  # [NJ, S, KC]
    KC = Pf.shape[2]
    cnt_P = Pf[:, 0, :]
    se_P = Pf[:, 1, :] + Pf[:, 2, :]

    kk = np.arange(KC)
    zero = np.zeros((1, KC))
    cum_cnt = np.concatenate([zero, np.cumsum(cnt_P, axis=0)], axis=0)  # [NJ+1, KC]
    cum_se = np.concatenate([zero, np.cumsum(se_P, axis=0)], axis=0)

    t2 = cols // 256
    pre_cnt = cum_cnt[t2, kk]
    pre_se = cum_se[t2, kk]
    suf_cnt = cum_cnt[NJ] - cum_cnt[t2 + 1, kk]
    suf_se = cum_se[NJ] - cum_se[t2 + 1, kk]

    # exact in-block window (256 rows around the diagonal crossover)
    d = (cols % 256).astype(np.int64)
    rows = (cols - d)[:, None] + np.arange(256)[None, :]     # [KC, 256]
    W = node_adj[rows, cols[:, None]] != 0
    dr = np.arange(256)[None, :]
    W &= dr != d[:, None]                                    # drop diagonal
    e_win = e[rows]
    wlow = W & (dr < d[:, None])
    wup = W & (dr > d[:, None])
    lower_cnt = pre_cnt + wlow.sum(1)
    upper_cnt = suf_cnt + wup.sum(1)
    lower_se = pre_se + (e_win * wlow).sum(1)
    upper_se = suf_se + (e_win * wup).sum(1)

    # exact positive-row stats (~2% of rows)
    prows = np.flatnonzero(np.asarray(targets).reshape(-1) != 0)
    Ap = node_adj[np.ix_(prows, cols)] != 0                  # [npos, KC]
    Ap &= prows[:, None] != cols[None, :]
    plow = prows[:, None] < cols[None, :]
    poscnt_low = (Ap & plow).sum(0)
    poscnt_up = (Ap & ~plow).sum(0)
    poslogit_low = (out[prows, None] * (Ap & plow)).sum(0)
    poslogit_up = (out[prows, None] * (Ap & ~plow)).sum(0)

    def side(cnt, se, poscnt, poslogit):
        valid = (poscnt == 1) & (cnt > 0.5)
        lse = np.log(np.where(valid, np.maximum(se, 1e-300), 1.0))
        return np.where(valid, (lse - poslogit) / np.maximum(cnt, 1.0), 0.0)

    contrib = side(lower_cnt, lower_se, poscnt_low, poslogit_low) + side(
        upper_cnt, upper_se, poscnt_up, poslogit_up
    )
    return np.float32(contrib[mapk].sum())


def kernel(outputs, targets, node_adj, idx_node, _simulate=False):
    node_adj = np.asarray(node_adj)
    idx = np.asarray(idx_node).reshape(-1).astype(np.int64)
    in_maps, slab, cols, mapk, e = _prepare(node_adj, idx, outputs)
    stats = _sim_stats(in_maps, slab) if _simulate else _device_stats(in_maps, slab)
    return _combine(stats, node_adj, outputs, targets, cols, mapk, e)
